# revision 26
# baseline (speedup 1.0000x reference)
"""CenterLossB kernel for 8 Trainium2 NeuronCores.

Data-parallel over the batch: each of the 8 cores processes 8192 rows of
feat/label/wei.  The loss

    own      = sum_i w_i * ||f_i - c_{l_i}||^2
    distocen = sum_i w_i * (||f_i - c_{(l_i+1)%3}||^2 + ||f_i - c_{(l_i+2)%3}||^2)
    out      = own * (1 + 1/distocen) / 2 / B

is rewritten algebraically so the device only produces small per-core
partials (centers never need to go to the device):

    A       = sum_i w_i ||f_i||^2                  (fp32, DVE fused mul+reduce)
    V[k,:]  = sum_i w_i 1[l_i=k] f_i   k=0..2      (PE matmul, PSUM fp32 accum)
    V[3,:]  = sum_i w_i f_i
    wsum[k] = sum_i w_i 1[l_i=k],  wsum[3] = sum_i w_i

Host combine (float64):
    own   = A - 2*sum_k c_k.V[k] + sum_k ||c_k||^2 wsum[k]
    total = 3A - 2*(sum_k c_k).V[3] + (sum_k ||c_k||^2) * wsum[3]
    distocen = total - own
"""

import os
from contextlib import ExitStack

import numpy as np

import concourse.bass as bass
import concourse.bacc as bacc
import concourse.tile as tile
from concourse import mybir
from concourse.bass_utils import run_bass_kernel_spmd

N_CORES = 8
B = 65536
D = 512
BC = B // N_CORES          # 8192 rows per core
P = 128                    # SBUF partitions
NT = BC // P               # 64 sub-tiles of 128 rows
CHUNK = 4                  # sub-tiles per DMA chunk (4*128 rows * 2KB = 1 MiB)
NCHUNK = NT // CHUNK       # 16 chunks

# Results of the last device run (for test harnesses to inspect timing).
LAST_RESULTS = None

_COMPILED = {}


def _build(reps=1, chunk=CHUNK, bufs=16, alt_dma=False, sizes=None,
           dma_only=False, staggered=False, split_stt=False, io_rings=False,
           no_stt=False, no_vout=False, bf16_trash=False, tmajor=False,
           act_num=0, act_den=16, out_eng="sync", trash_psum=False,
           no_mm=False, fake_elem=False, felem_num=None, felem_den=16):
    f32 = mybir.dt.float32
    f32r = mybir.dt.float32r
    i32 = mybir.dt.int32
    X = mybir.AxisListType.X
    op = mybir.AluOpType

    nc = bacc.Bacc("TRN2", target_bir_lowering=False, debug=False,
                   num_devices=N_CORES)

    # feat is declared float32r (same 32-bit layout as fp32; numpy side is
    # np.float32) so the full-rate fp32r matmul path passes BIR verification:
    # the verifier requires producers of fp32r matmul operands to be
    # fp32r-typed.  DVE consumers bitcast back to plain fp32.
    feat_d = nc.dram_tensor("feat", [BC, D], f32r, kind="ExternalInput")
    lab_d = nc.dram_tensor("label", [BC], i32, kind="ExternalInput")
    wei_d = nc.dram_tensor("wei", [BC], f32, kind="ExternalInput")
    V_d = nc.dram_tensor("V", [4, D], f32, kind="ExternalOutput")
    S_d = nc.dram_tensor("S", [5, 1], f32, kind="ExternalOutput")

    # row layout: global row r = p*NT + t  (p = partition, t = sub-tile idx)
    # tmajor (diagnostic, dma_only benches only): r = t*P + p, which makes each
    # chunk DMA one contiguous DRAM span instead of 128 scattered blocks
    if tmajor:
        assert dma_only, "tmajor needs wei/label transpose plumbing for compute"
        feat_r = feat_d.ap().rearrange("(t p) d -> p t d", p=P)
    else:
        feat_r = feat_d.ap().rearrange("(p t) d -> p t d", p=P)
    lab_r = lab_d.ap().rearrange("(p t) -> p t", p=P)
    wei_r = wei_d.ap().rearrange("(p t) -> p t", p=P)

    if sizes is None:
        sizes = (chunk,) * (NT // chunk)
    assert sum(sizes) == NT
    starts = [sum(sizes[:i]) for i in range(len(sizes))]
    max_chunk = max(sizes)

    with tile.TileContext(nc) as tc, ExitStack() as ctx:
        singles = ctx.enter_context(tc.tile_pool(name="singles", bufs=1))
        feat_pool = ctx.enter_context(tc.tile_pool(name="feat", bufs=bufs))
        psum = ctx.enter_context(tc.tile_pool(name="psum", bufs=1, space="PSUM"))

        # small I/O rides the second HWDGE ring (qActDynamicHW) so the feat
        # stream on qSPDynamicHW starts without queuing behind it
        io_eng = nc.scalar if io_rings else nc.sync
        w_all = singles.tile([P, NT], f32)
        io_eng.dma_start(w_all[:], wei_r)
        lab_i = singles.tile([P, NT], i32)
        io_eng.dma_start(lab_i[:], lab_r)
        lab_f = singles.tile([P, NT], f32)
        nc.vector.tensor_copy(lab_f[:], lab_i[:])

        # wk[p, k, t] = w * 1[label == k] for k=0..2; wk[p, 3, t] = w
        wk = singles.tile([P, 4, NT], f32)
        for k in range(3):
            nc.vector.scalar_tensor_tensor(
                out=wk[:, k, :], in0=lab_f[:], scalar=float(k), in1=w_all[:],
                op0=op.is_equal, op1=op.mult)
        nc.vector.tensor_copy(wk[:, 3, :], w_all[:])
        # bit-preserving copy into an fp32r-typed tile for the matmul lhsT
        wk_r = singles.tile([P, 4, NT], f32r)
        nc.vector.tensor_copy(wk_r[:], wk[:].bitcast(f32r))
        wk_t = wk_r.rearrange("p k t -> p t k")

        swsum = singles.tile([P, NT], f32)
        tdt = mybir.dt.bfloat16 if bf16_trash else f32
        if trash_psum:
            trash = psum.tile([P, D], f32, tag="trash")
            trash2 = psum.tile([P, D], f32, tag="trash2")
        else:
            trash = singles.tile([P, D], tdt)
            trash2 = singles.tile([P, D], tdt)
        ones = singles.tile([P, 1], f32)
        nc.vector.memset(ones[:], 1.0)

        fsrc = fsrc2 = None
        if fake_elem or felem_num is not None:
            fsrc = singles.tile([P, D], f32)
            fsrc2 = singles.tile([P, D], f32)
            nc.vector.memset(fsrc[:], 1.0)
            nc.vector.memset(fsrc2[:], 1.0)

        # sqrt(w) lets the ACT engine compute w*f^2 as Square(sqrt(w)*f)
        # with per-partition scale + free-dim accumulation, offloading part
        # of the elementwise pass from DVE.
        sqrtw = None
        if act_num:
            sqrtw = singles.tile([P, NT], f32)
            nc.scalar.activation(sqrtw[:], w_all[:],
                                 mybir.ActivationFunctionType.Sqrt)

        # rs[:, 0:4] (per-partition wk sums) is loop-invariant: compute once.
        rs = singles.tile([P, 5], f32)
        for k in range(4):
            nc.vector.tensor_reduce(out=rs[:, k:k + 1], in_=wk[:, k, :],
                                    axis=X, op=op.add)

        def body():
            psum_V = None
            if not (dma_only or no_mm):
                psum_V = psum.tile([4, D], f32, tag="psum_V")
            for c, (t0, sz) in enumerate(zip(starts, sizes)):
                F = feat_pool.tile([P, max_chunk, D], f32r)
                eng = nc.scalar if (alt_dma and c % 2) else nc.sync
                eng.dma_start(F[:, :sz, :], feat_r[:, t0:t0 + sz, :])
                for a in range(sz if not dma_only else 0):
                    t = t0 + a
                    if not no_mm:
                        nc.tensor.matmul(
                            psum_V[:],
                            wk_t[:, t, :],
                            F[:, a, :],
                            start=(t == 0), stop=(t == NT - 1))
                    if no_stt:
                        continue
                    # fake_elem: same elementwise engine load, but reading a
                    # resident tile instead of the DMA'd chunk (diagnostic)
                    fake_t = fake_elem or (
                        felem_num is not None
                        and (t % felem_den) >= felem_num)
                    if act_num and ((t * act_num) % act_den) < act_num:
                        src = fsrc2[:] if fake_t else F[:, a, :].bitcast(f32)
                        nc.scalar.activation(
                            out=trash2[:], in_=src,
                            func=mybir.ActivationFunctionType.Square,
                            scale=sqrtw[:, t:t + 1],
                            accum_out=swsum[:, t:t + 1])
                        continue
                    if split_stt and t % 2:
                        eng_v, tr = nc.gpsimd, trash2
                    else:
                        eng_v, tr = nc.vector, trash
                    src = fsrc[:] if fake_t else F[:, a, :].bitcast(f32)
                    eng_v.scalar_tensor_tensor(
                        out=tr[:], in0=src,
                        scalar=w_all[:, t:t + 1],
                        in1=src, op0=op.mult, op1=op.mult,
                        accum_out=swsum[:, t:t + 1])

            # epilogue: cross-partition reduction via matmul
            if dma_only or no_stt:
                nc.vector.memset(rs[:, 4:5], 0.0)
            else:
                nc.vector.tensor_reduce(out=rs[:, 4:5], in_=swsum[:], axis=X,
                                        op=op.add)
            psum_S = psum.tile([5, 1], f32)
            nc.tensor.matmul(psum_S[:], rs[:], ones[:])

            # output DMAs ride their own ring so the feat stream on the sync
            # ring never waits behind them (HWDGE rings are FIFO per engine)
            oe = {"sync": nc.sync, "scalar": nc.scalar,
                  "gpsimd": nc.gpsimd}[out_eng]
            v_sb = singles.tile([4, D], f32)
            if dma_only or no_vout or no_mm:
                nc.vector.memset(v_sb[:], 0.0)
            else:
                nc.vector.tensor_copy(v_sb[:], psum_V[:])
            oe.dma_start(V_d.ap(), v_sb[:])
            s_sb = singles.tile([5, 1], f32)
            nc.vector.tensor_copy(s_sb[:], psum_S[:])
            oe.dma_start(S_d.ap(), s_sb[:])

        if reps == 1:
            body()
        else:
            with tc.For_i(0, reps, 1, staggered_reset=staggered):
                body()

    nc.compile()
    return nc


def _build2(reps=1, sizes=(4,) * 14 + (2, 2, 2, 2), nact_head=26, vsplit=56,
            bufs=None, tail_scalar=True, tail_sw_sync=False):
    """Tail-optimized variant.

    - chunks taper at the end so the last compute dependency is small
    - V accumulates in two PSUM banks: tiles [0, vsplit) drain to HBM
      mid-stream, only tiles [vsplit, NT) remain in the tail
    - per-partition sums (rs, swsum) ship raw to the host, which does the
      cross-partition reduction: no on-device epilogue matmul
    - elementwise pass split DVE/ACT (dithered, ACT-free tail)
    - prologue w/label DMAs ride the scalar ring so the feat stream on the
      sync ring starts immediately
    """
    f32 = mybir.dt.float32
    f32r = mybir.dt.float32r
    i32 = mybir.dt.int32
    X = mybir.AxisListType.X
    op = mybir.AluOpType

    assert sum(sizes) == NT
    starts = [sum(sizes[:i]) for i in range(len(sizes))]
    max_chunk = max(sizes)
    if bufs is None:
        bufs = len(sizes)

    # engine map: tiles >= vsplit go to DVE (faster per tile, keeps the ACT
    # engine free for the V-tail PSUM copy); head tiles dithered
    n_head = vsplit
    use_act = [False] * NT
    acc = 0
    for t in range(n_head):
        acc += nact_head
        if acc >= n_head:
            acc -= n_head
            use_act[t] = True

    nc = bacc.Bacc("TRN2", target_bir_lowering=False, debug=False,
                   num_devices=N_CORES)

    feat_d = nc.dram_tensor("feat", [BC, D], f32r, kind="ExternalInput")
    lab_d = nc.dram_tensor("label", [BC], i32, kind="ExternalInput")
    wei_d = nc.dram_tensor("wei", [BC], f32, kind="ExternalInput")
    V_d = nc.dram_tensor("V", [8, D], f32, kind="ExternalOutput")
    S1_d = nc.dram_tensor("S1", [P, 4], f32, kind="ExternalOutput")
    SW_d = nc.dram_tensor("SW", [P, NT], f32, kind="ExternalOutput")

    feat_r = feat_d.ap().rearrange("(p t) d -> p t d", p=P)
    lab_r = lab_d.ap().rearrange("(p t) -> p t", p=P)
    wei_r = wei_d.ap().rearrange("(p t) -> p t", p=P)

    with tile.TileContext(nc) as tc, ExitStack() as ctx:
        singles = ctx.enter_context(tc.tile_pool(name="singles", bufs=1))
        feat_pool = ctx.enter_context(tc.tile_pool(name="feat", bufs=bufs))
        psum = ctx.enter_context(tc.tile_pool(name="psum", bufs=1, space="PSUM"))

        w_all = singles.tile([P, NT], f32)
        nc.scalar.dma_start(w_all[:], wei_r)
        lab_i = singles.tile([P, NT], i32)
        nc.scalar.dma_start(lab_i[:], lab_r)
        lab_f = singles.tile([P, NT], f32)
        nc.vector.tensor_copy(lab_f[:], lab_i[:])

        # wk[p, k, t] = w * 1[label == k] for k=0..2; wk[p, 3, t] = w
        wk = singles.tile([P, 4, NT], f32)
        for k in range(3):
            nc.vector.scalar_tensor_tensor(
                out=wk[:, k, :], in0=lab_f[:], scalar=float(k), in1=w_all[:],
                op0=op.is_equal, op1=op.mult)
        nc.vector.tensor_copy(wk[:, 3, :], w_all[:])
        wk_r = singles.tile([P, 4, NT], f32r)
        nc.vector.tensor_copy(wk_r[:], wk[:].bitcast(f32r))
        wk_t = wk_r.rearrange("p k t -> p t k")

        sqrtw = singles.tile([P, NT], f32)
        nc.scalar.activation(sqrtw[:], w_all[:],
                             mybir.ActivationFunctionType.Sqrt)

        # loop-invariant per-partition wk sums: DMA out once, host reduces
        rs = singles.tile([P, 4], f32)
        for k in range(4):
            nc.vector.tensor_reduce(out=rs[:, k:k + 1], in_=wk[:, k, :],
                                    axis=X, op=op.add)
        nc.gpsimd.dma_start(S1_d.ap(), rs[:])

        swsum = singles.tile([P, NT], f32)
        trash = singles.tile([P, D], f32)
        trash2 = singles.tile([P, D], f32)

        def body():
            psum_Va = psum.tile([4, D], f32, tag="psum_Va")
            psum_Vb = psum.tile([4, D], f32, tag="psum_Vb")
            for c, (t0, sz) in enumerate(zip(starts, sizes)):
                F = feat_pool.tile([P, max_chunk, D], f32r)
                nc.sync.dma_start(F[:, :sz, :], feat_r[:, t0:t0 + sz, :])
                for a in range(sz):
                    t = t0 + a
                    pv = psum_Va if t < vsplit else psum_Vb
                    nc.tensor.matmul(
                        pv[:],
                        wk_t[:, t, :],
                        F[:, a, :],
                        start=(t == 0 or t == vsplit),
                        stop=(t == vsplit - 1 or t == NT - 1))
                    if use_act[t]:
                        nc.scalar.activation(
                            out=trash2[:], in_=F[:, a, :].bitcast(f32),
                            func=mybir.ActivationFunctionType.Square,
                            scale=sqrtw[:, t:t + 1],
                            accum_out=swsum[:, t:t + 1])
                    else:
                        nc.vector.scalar_tensor_tensor(
                            out=trash[:], in0=F[:, a, :].bitcast(f32),
                            scalar=w_all[:, t:t + 1],
                            in1=F[:, a, :].bitcast(f32),
                            op0=op.mult, op1=op.mult,
                            accum_out=swsum[:, t:t + 1])
                    if t == vsplit - 1:
                        # early V half + early swsum columns drain to HBM
                        # while the remaining chunks still stream
                        va_sb = singles.tile([4, D], f32)
                        nc.vector.tensor_copy(va_sb[:], psum_Va[:])
                        nc.gpsimd.dma_start(V_d.ap()[0:4, :], va_sb[:])
                        nc.gpsimd.dma_start(SW_d.ap()[:, 0:vsplit],
                                            swsum[:, 0:vsplit])

            # tail: only the late V bank and the last swsum columns remain
            vb_sb = singles.tile([4, D], f32)
            nc.scalar.copy(vb_sb[:], psum_Vb[:])
            oe = nc.scalar if tail_scalar else nc.gpsimd
            swe = nc.sync if tail_sw_sync else oe
            swe.dma_start(SW_d.ap()[:, vsplit:NT], swsum[:, vsplit:NT])
            oe.dma_start(V_d.ap()[4:8, :], vb_sb[:])

        if reps == 1:
            body()
        else:
            with tc.For_i(0, reps, 1):
                body()

    nc.compile()
    return nc


def _get_compiled(reps=1, v2=False, **kw):
    key = (reps, v2, tuple(sorted(kw.items())))
    if key not in _COMPILED:
        _COMPILED[key] = (_build2 if v2 else _build)(reps, **kw)
    return _COMPILED[key]


# Tuned configuration used by kernel() and by test.py's benchmark fallback.
TUNED_KW = dict(v2=True, tail_sw_sync=True, vsplit=60, nact_head=22,
                sizes=(4,) * 14 + (2, 2, 2, 1, 1))


def kernel(feat, label, wei, centers, batch_size):
    global LAST_RESULTS
    feat = np.ascontiguousarray(np.asarray(feat, dtype=np.float32))
    label = np.ascontiguousarray(np.asarray(label, dtype=np.int32))
    wei = np.ascontiguousarray(np.asarray(wei, dtype=np.float32))
    centers = np.asarray(centers, dtype=np.float32)
    bsz = float(np.asarray(batch_size))

    nc = _get_compiled(**TUNED_KW)

    in_maps = []
    for i in range(N_CORES):
        sl = slice(i * BC, (i + 1) * BC)
        in_maps.append({
            "feat": feat[sl],
            "label": label[sl],
            "wei": wei[sl],
        })

    try:
        res = run_bass_kernel_spmd(nc, in_maps, list(range(N_CORES)))
    except ModuleNotFoundError:
        # BASS_TRACE was requested but this environment lacks the axon NTFF
        # profile hook (antenv.axon_hooks) — rerun without tracing.
        prev = os.environ.get("BASS_NEVER_TRACE")
        os.environ["BASS_NEVER_TRACE"] = "1"
        try:
            res = run_bass_kernel_spmd(nc, in_maps, list(range(N_CORES)))
        finally:
            if prev is None:
                os.environ.pop("BASS_NEVER_TRACE", None)
            else:
                os.environ["BASS_NEVER_TRACE"] = prev
    LAST_RESULTS = res

    # host combine in float64: cross-partition reductions of the raw
    # per-partition partials, then the small closed-form assembly
    c = centers.astype(np.float64)            # [3, D]
    cn = (c * c).sum(axis=1)                  # ||c_k||^2
    csum = cn.sum()
    s_cent = c.sum(axis=0)                    # sum_k c_k

    A = 0.0
    B2 = 0.0
    T2 = 0.0
    wsum = np.zeros(4, dtype=np.float64)
    for r in res.results:
        V8 = r["V"].astype(np.float64)        # [8, D] (early + late banks)
        V = V8[0:4] + V8[4:8]
        S1 = r["S1"].astype(np.float64)       # [P, 4] per-partition wk sums
        SW = r["SW"].astype(np.float64)       # [P, NT] per-(p,t) w*|f|^2
        B2 += float((c * V[:3]).sum())
        T2 += float((s_cent * V[3]).sum())
        wsum += S1.sum(axis=0)
        A += float(SW.sum())

    own = A - 2.0 * B2 + float((cn * wsum[:3]).sum())
    total = 3.0 * A - 2.0 * T2 + csum * wsum[3]
    distocen = total - own
    out = own * (1.0 + 1.0 / distocen) / 2.0 / bsz
    return np.float32(out)

